# revision 1
# baseline (speedup 1.0000x reference)
"""Causal self-attention on 8 TRN2 NeuronCores.

Sharding: core c handles batch b = c//2 and head-group g = c%2 (8 of 16 heads).
Each core computes its partial y^T = w_proj[slice].T @ o^T (contraction over its
512 o-channels); the host sums the two partials per batch and adds b_proj.

Shapes (hardcoded): B=4, T=2048, C=1024, H=16, HD=64.
"""

import numpy as np

B, T, C, H = 4, 2048, 1024, 16
HD = C // H          # 64
G = 2                # head groups
NHL = H // G         # 8 heads per core
GQ = NHL * HD        # 512 channel slice per core
P = 128
NT = T // P          # 16 token tiles / k-chunks
NCHUNK = C // P      # 8 contraction chunks for qkv
SCALE = 1.0 / float(np.sqrt(HD))

_PROGRAM = None


def _emit(ctx, tc, aps, mybir, bass):
    import contextlib

    nc = tc.nc
    f32 = mybir.dt.float32
    f32r = mybir.dt.float32r
    bf16 = mybir.dt.bfloat16
    EXP = mybir.ActivationFunctionType.Exp

    x_d, wqkv_d, bqk_d, bv_d, wp_d, yT_d = (
        aps["x"], aps["wqkv"], aps["bqk"], aps["bv"], aps["wp"], aps["yT"],
    )

    # ---------------- pools ----------------
    const = ctx.enter_context(tc.tile_pool(name="const", bufs=1))
    dramp = ctx.enter_context(tc.tile_pool(name="dramp", bufs=1, space="DRAM"))
    # psum: main 2x[128,1024] (4 banks) + pv 4x[128,512] (4 banks)
    ps_main = ctx.enter_context(tc.tile_pool(name="ps_main", bufs=2, space="PSUM"))
    ps_pv = ctx.enter_context(tc.tile_pool(name="ps_pv", bufs=4, space="PSUM"))

    qkp = ctx.enter_context(tc.tile_pool(name="qkp", bufs=8))
    vap = ctx.enter_context(tc.tile_pool(name="vap", bufs=16))
    ptp = ctx.enter_context(tc.tile_pool(name="ptp", bufs=3))
    otp = ctx.enter_context(tc.tile_pool(name="otp", bufs=2))
    rcp = ctx.enter_context(tc.tile_pool(name="rcp", bufs=2))

    # constants
    identity = const.tile([P, P], f32)
    from concourse.masks import make_identity
    make_identity(nc, identity)
    bqk_sb = const.tile([P, 8], f32)
    nc.sync.dma_start(bqk_sb[:], bqk_d[:])
    bvb = const.tile([P, GQ], f32)
    nc.sync.dma_start(bvb[:], bv_d[None, :].to_broadcast((P, GQ)))
    ones8 = const.tile([P, NHL, 1], f32)
    nc.vector.memset(ones8[:], 1.0)

    odram = dramp.tile([GQ, T], f32r, space="DRAM")

    # ---------------- phase A: load x, build xT ----------------
    stackAB = contextlib.ExitStack()
    xTp = stackAB.enter_context(tc.tile_pool(name="xTp", bufs=8))
    wqkp = stackAB.enter_context(tc.tile_pool(name="wqkp", bufs=4))
    wvp = stackAB.enter_context(tc.tile_pool(name="wvp", bufs=1))
    stackA = contextlib.ExitStack()
    xp = stackA.enter_context(tc.tile_pool(name="xp", bufs=2))

    xT = []  # 8 tiles [128 c, 2048 t]
    for r in range(NCHUNK):
        t_ = xTp.tile([P, T], f32r, name=f"xT{r}", tag="xT")
        xT.append(t_)

    for tg in range(NT // 2):  # groups of 2 t-tiles
        xts = []
        for tt in range(2):
            t = 2 * tg + tt
            x_t = xp.tile([P, C], f32, name=f"x_{t}", tag="x")
            nc.sync.dma_start(x_t[:], x_d[t * P:(t + 1) * P, :])
            xts.append(x_t)
        for r in range(NCHUNK):
            tp = ps_main.tile([P, 256], f32, name=f"tp_{tg}_{r}", tag="main")
            for tt in range(2):
                nc.tensor.transpose(
                    tp[:, tt * P:(tt + 1) * P],
                    xts[tt][:, r * P:(r + 1) * P],
                    identity,
                )
            nc.vector.tensor_copy(xT[r][:, tg * 256:(tg + 1) * 256], tp[:])
    stackA.close()

    # ---------------- phase B: qkv ----------------
    qkT = []  # bf16 tiles [128 c', 2048 t]; 0..3 = qT, 4..7 = kT
    for ct in range(8):
        o_t = qkp.tile([P, T], bf16, name=f"qkT{ct}", tag="qkT")
        qkT.append(o_t)

    vaug = []  # [128 k, 8 heads, 65] per k-chunk (col 64 = ones for denom)
    for t in range(NT):
        va = vap.tile([P, NHL, HD + 1], f32r, name=f"vaug{t}", tag="vaug")
        nc.vector.tensor_copy(va[:, :, HD:HD + 1], ones8[:])
        vaug.append(va)

    wqkv_r = wqkv_d.rearrange("(a p) n -> p a n", p=P)  # [128, 8, 1536]

    wqk_tiles = {}

    def emit_qk_half(ct, twp):
        # one [128,1024] half of output tile ct (q cols twp*1024..)
        if ct not in wqk_tiles:
            col0 = ct * P
            w_t = wqkp.tile([P, NCHUNK, P], f32r, name=f"wqk_{ct}", tag="wqk")
            nc.sync.dma_start(w_t[:], wqkv_r[:, :, col0:col0 + P])
            wqk_tiles[ct] = w_t
        w_t = wqk_tiles[ct]
        ps = ps_main.tile([P, 1024], f32, name=f"qkps_{ct}_{twp}", tag="main")
        for a in range(NCHUNK):
            for sw in range(2):
                nc.tensor.matmul(
                    ps[:, sw * 512:(sw + 1) * 512],
                    w_t[:, a, :],
                    xT[a][:, twp * 1024 + sw * 512:twp * 1024 + (sw + 1) * 512],
                    start=(a == 0),
                    stop=(a == NCHUNK - 1),
                )
        nc.vector.tensor_scalar_add(
            qkT[ct][:, twp * 1024:(twp + 1) * 1024], ps[:], bqk_sb[:, ct:ct + 1]
        )

    wv_tile = {}

    def emit_v(t0, t1):
        if not wv_tile:
            w_t = wvp.tile([P, NCHUNK, GQ], f32r, name="wv", tag="wv")
            nc.sync.dma_start(w_t[:], wqkv_r[:, :, 2 * GQ:3 * GQ])
            wv_tile[0] = w_t
        w_t = wv_tile[0]
        for t in range(t0, t1):
            ps = ps_pv.tile([P, GQ], f32, name=f"vps_{t}", tag="ps_pv")
            for a in range(NCHUNK):
                nc.tensor.matmul(
                    ps[:],
                    xT[a][:, t * P:(t + 1) * P],
                    w_t[:, a, :],
                    start=(a == 0),
                    stop=(a == NCHUNK - 1),
                )
            nc.vector.tensor_add(
                vaug[t][:, :, 0:HD],
                ps[:].rearrange("p (h d) -> p h d", h=NHL),
                bvb[:].rearrange("p (h d) -> p h d", h=NHL),
            )

    # ---------------- phase C: attention ----------------
    # Head pairs: head A on PE row strip 0, head B on strip 64; score pieces
    # for the two heads live in the two banks of one [128,1024] psum tile, so
    # the row-packed matmuls run concurrently and one exp covers both heads.
    # Quarter-outer loop: each 512-wide q-window accumulates PV fully, then
    # normalizes while the next window runs (pv pool rotation hides it).
    def attn_pair(hp, extra=None):
        qt = qkT[hp]
        kt = qkT[4 + hp]
        for m in range(4):  # quarter windows of 512 q
            if extra and m in extra:
                for fn in extra[m]:
                    fn()
            ws = m * 512
            pvt = {}
            for hh in range(2):
                pvt[hh] = ps_pv.tile(
                    [P, 512], f32, name=f"pv_{hp}_{m}_{hh}", tag="ps_pv")
            for i in range(4 * m + 4):  # causal k-chunks for this window
                s = max(i * P, ws)
                o = s - ws
                # head A piece in cols [o, 512), head B in [512, 1024-o)
                sc = ps_main.tile([P, 1024], f32, name=f"sc_{hp}_{m}_{i}",
                                  tag="main")
                for hh in range(2):
                    r0 = hh * HD
                    c0 = o if hh == 0 else 512
                    nc.tensor.matmul(
                        sc[:, c0:c0 + 512 - o],
                        kt[r0:r0 + HD, i * P:(i + 1) * P],
                        qt[r0:r0 + HD, s:ws + 512],
                        start=True,
                        stop=True,
                    )
                pt = ptp.tile([P, 1024], f32r, name=f"pt_{hp}_{m}_{i}",
                              tag="pt")
                nc.scalar.activation(pt[:, o:1024 - o], sc[:, o:1024 - o],
                                     EXP, scale=SCALE)
                diag = i * P >= ws
                for hh in range(2):
                    c0 = o if hh == 0 else 512
                    if diag:
                        nc.gpsimd.affine_select(
                            out=pt[:, c0:c0 + P],
                            in_=pt[:, c0:c0 + P],
                            compare_op=mybir.AluOpType.is_ge,
                            fill=0.0,
                            base=0,
                            pattern=[[1, P]],
                            channel_multiplier=-1,
                        )
                    nc.tensor.matmul(
                        pvt[hh][0:HD + 1, o:],
                        vaug[i][:, 2 * hp + hh, :],
                        pt[:, c0:c0 + 512 - o],
                        start=(i == 0),
                        stop=(i == 4 * m + 3),
                    )
            # normalize both heads: ot rows 0:64 = head A, 64:128 = head B
            ot = otp.tile([P, 512], f32r, name=f"ot_{hp}_{m}",
                          tag=("ot3" if hp == 3 else "ot"),
                          bufs=(4 if hp == 3 else None))
            for hh in range(2):
                rc = rcp.tile([1, 512], f32, name=f"rc_{hp}_{m}_{hh}", tag="rc")
                nc.vector.reciprocal(rc[:], pvt[hh][HD:HD + 1, :])
                rcb = rcp.tile([HD, 512], f32, name=f"rcb_{hp}_{m}_{hh}",
                               tag="rcb")
                nc.gpsimd.partition_broadcast(rcb[:], rc[:])
                nc.vector.tensor_mul(
                    ot[hh * HD:(hh + 1) * HD, :], pvt[hh][0:HD, :], rcb[:])
            if hp == 3:
                ot3.append(ot)  # stays in SBUF, feeds proj directly
            else:
                nc.sync.dma_start(odram[hp * P:(hp + 1) * P, ws:ws + 512], ot[:])

    # interleave qkv production with attention so the PE stream stays dense
    # through the ACT-bound attention stretches (keeps HAM warm); emit only
    # what pair 0 quarter 0 needs before starting it.
    ot3 = []
    emit_v(0, 4)
    emit_qk_half(0, 0)
    emit_qk_half(4, 0)
    emit_v(4, 16)
    emit_qk_half(0, 1)
    emit_qk_half(4, 1)
    attn_pair(0)
    for j in range(1, 4):
        for twp in range(2):
            emit_qk_half(j, twp)
            emit_qk_half(4 + j, twp)
        attn_pair(j)

    stackAB.close()  # release x / w / xT pools

    # ---------------- phase D: proj ----------------
    stackD = contextlib.ExitStack()
    orp = stackD.enter_context(tc.tile_pool(name="orp", bufs=3))
    wpp = stackD.enter_context(tc.tile_pool(name="wpp", bufs=1))
    ysp = stackD.enter_context(tc.tile_pool(name="ysp", bufs=3))

    wp_t = wpp.tile([P, 4, C], f32r, name="wp", tag="wp")
    nc.sync.dma_start(wp_t[:], wp_d.rearrange("(a p) n -> p a n", p=P))
    oTr = []
    for a in range(3):
        o_t = orp.tile([P, T], f32r, name=f"oTr{a}", tag="oTr")
        nc.sync.dma_start(o_t[:], odram[a * P:(a + 1) * P, :])
        oTr.append(o_t)
    for mt in range(NCHUNK):  # cout tiles
        for twp in range(2):
            ps = ps_main.tile([P, 1024], f32, name=f"yps_{mt}_{twp}", tag="main")
            for a in range(4):
                for sw in range(2):
                    rhs = (oTr[a][:, twp * 1024 + sw * 512:twp * 1024 + (sw + 1) * 512]
                           if a < 3 else ot3[2 * twp + sw][:, :])
                    nc.tensor.matmul(
                        ps[:, sw * 512:(sw + 1) * 512],
                        wp_t[:, a, mt * P:(mt + 1) * P],
                        rhs,
                        start=(a == 0),
                        stop=(a == 3),
                    )
            ys = ysp.tile([P, 1024], f32, name=f"ys_{mt}_{twp}", tag="ys")
            nc.scalar.copy(ys[:], ps[:])
            nc.sync.dma_start(
                yT_d[mt * P:(mt + 1) * P, twp * 1024:(twp + 1) * 1024], ys[:]
            )
    stackD.close()


def _build_program():
    import contextlib

    import concourse.bass as bass
    import concourse.mybir as mybir
    import concourse.tile as tile
    from concourse import bacc

    nc = bacc.Bacc("TRN2", target_bir_lowering=False, debug=False, num_devices=8)
    f32 = mybir.dt.float32
    aps = {
        "x": nc.dram_tensor("x", [T, C], f32, kind="ExternalInput").ap(),
        "wqkv": nc.dram_tensor("wqkv", [C, 3 * GQ], mybir.dt.float32r, kind="ExternalInput").ap(),
        "bqk": nc.dram_tensor("bqk", [P, 8], f32, kind="ExternalInput").ap(),
        "bv": nc.dram_tensor("bv", [GQ], f32, kind="ExternalInput").ap(),
        "wp": nc.dram_tensor("wp", [GQ, C], mybir.dt.float32r, kind="ExternalInput").ap(),
        "yT": nc.dram_tensor("yT", [C, T], f32, kind="ExternalOutput").ap(),
    }
    with tile.TileContext(nc) as tc:
        with contextlib.ExitStack() as ctx:
            _emit(ctx, tc, aps, mybir, bass)
    nc.compile()
    return nc


def get_program():
    global _PROGRAM
    if _PROGRAM is None:
        _PROGRAM = _build_program()
    return _PROGRAM


def make_in_maps(x, w_qkv, b_qkv, w_proj):
    x = np.asarray(x, np.float32)
    w_qkv = np.asarray(w_qkv, np.float32)
    b_qkv = np.asarray(b_qkv, np.float32)
    w_proj = np.asarray(w_proj, np.float32)
    in_maps = []
    for c in range(8):
        b = c // 2
        g = c % 2
        q0 = g * GQ
        wq = w_qkv[:, q0:q0 + GQ]
        wk = w_qkv[:, C + q0:C + q0 + GQ]
        wv = w_qkv[:, 2 * C + q0:2 * C + q0 + GQ]
        wqkv = np.ascontiguousarray(np.concatenate([wq, wk, wv], axis=1))
        bq = b_qkv[q0:q0 + GQ]
        bk = b_qkv[C + q0:C + q0 + GQ]
        bqk = np.ascontiguousarray(np.concatenate([bq, bk]).reshape(8, P).T)
        bv = np.ascontiguousarray(b_qkv[2 * C + q0:2 * C + q0 + GQ])
        in_maps.append({
            "x": np.ascontiguousarray(x[b]),
            "wqkv": wqkv,
            "bqk": bqk,
            "bv": bv,
            "wp": np.ascontiguousarray(w_proj[q0:q0 + GQ, :]),
        })
    return in_maps


def combine_outputs(outs, b_proj):
    b_proj = np.asarray(b_proj, np.float32)
    y = np.empty((B, T, C), np.float32)
    for b in range(B):
        acc = outs[2 * b] + outs[2 * b + 1]  # [C, T]
        y[b] = acc.T + b_proj
    return y


def kernel(x, w_qkv, b_qkv, w_proj, b_proj, _trace=False):
    from concourse import bass_utils

    nc = get_program()
    in_maps = make_in_maps(x, w_qkv, b_qkv, w_proj)
    res = bass_utils.run_bass_kernel_spmd(
        nc, in_maps, core_ids=list(range(8)), trace=_trace
    )
    outs = [r["yT"] for r in res.results]
    y = combine_outputs(outs, b_proj)
    if _trace:
        return y, res
    return y



# revision 4
# speedup vs baseline: 1.0920x; 1.0920x over previous
"""Causal self-attention on 8 TRN2 NeuronCores.

Sharding: core c handles batch b = c//2 and head-group g = c%2 (8 of 16 heads).
Each core computes its partial y^T = w_proj[slice].T @ o^T (contraction over its
512 o-channels); the host sums the two partials per batch and adds b_proj.

Shapes (hardcoded): B=4, T=2048, C=1024, H=16, HD=64.

All matmul operands are bf16 (x/w_qkv/w_proj cast on host); accumulation is
fp32 in PSUM. x^T is loaded straight from DRAM with the xbar transpose DMA.
o stays in SBUF (bf16) and feeds proj directly.

Schedule: attention is ACT(exp)-bound, so qkv/v/proj work is emitted in small
units (one 512-wide psum accumulation each) interleaved between attention
chunks, keeping the PE stream dense while ACT crunches exp. proj for window m
runs inside pair 3 right after (3, m) completes. Diagonal causal masking is a
DVE multiply with a tril mask (gpsimd affine_select is broken for bf16 on HW).

PSUM: ps_main 2x[128,1024] holds score tiles AND all interleaved filler
accumulators (rotation deps always point backward in program order); ps_pv
4x[128,512] holds ONLY the per-window PV accumulators so window-to-window
overlap survives.
"""

import numpy as np

B, T, C, H = 4, 2048, 1024, 16
HD = C // H          # 64
G = 2                # head groups
NHL = H // G         # 8 heads per core
GQ = NHL * HD        # 512 channel slice per core
P = 128
NT = T // P          # 16 token tiles / k-chunks
NCHUNK = C // P      # 8 contraction chunks for qkv
SCALE = 1.0 / float(np.sqrt(HD))

_PROGRAM = None


def _emit(ctx, tc, aps, mybir, bass):
    import contextlib

    nc = tc.nc
    f32 = mybir.dt.float32
    bf16 = mybir.dt.bfloat16
    EXP = mybir.ActivationFunctionType.Exp

    x_d, wqkv_d, bqk_d, bv_d, wp_d, yT_d = (
        aps["x"], aps["wqkv"], aps["bqk"], aps["bv"], aps["wp"], aps["yT"],
    )

    # ---------------- pools ----------------
    const = ctx.enter_context(tc.tile_pool(name="const", bufs=1))
    ps_main = ctx.enter_context(tc.tile_pool(name="ps_main", bufs=2, space="PSUM"))
    ps_pv = ctx.enter_context(tc.tile_pool(name="ps_pv", bufs=4, space="PSUM"))

    qkp = ctx.enter_context(tc.tile_pool(name="qkp", bufs=8))
    vap = ctx.enter_context(tc.tile_pool(name="vap", bufs=16))
    ptp = ctx.enter_context(tc.tile_pool(name="ptp", bufs=3))
    otp = ctx.enter_context(tc.tile_pool(name="otp", bufs=16))
    rcp = ctx.enter_context(tc.tile_pool(name="rcp", bufs=2))
    xTp = ctx.enter_context(tc.tile_pool(name="xTp", bufs=8))
    wqkp = ctx.enter_context(tc.tile_pool(name="wqkp", bufs=4))
    wvp = ctx.enter_context(tc.tile_pool(name="wvp", bufs=1))
    wpp = ctx.enter_context(tc.tile_pool(name="wpp", bufs=1))
    ysp = ctx.enter_context(tc.tile_pool(name="ysp", bufs=3))

    # constants
    bqk_sb = const.tile([P, 8], f32)
    nc.sync.dma_start(bqk_sb[:], bqk_d[:])
    bvb = const.tile([P, GQ], f32)
    nc.sync.dma_start(bvb[:], bv_d[None, :].to_broadcast((P, GQ)))
    ones8 = const.tile([P, NHL, 1], f32)
    nc.vector.memset(ones8[:], 1.0)
    # tril causal mask, bf16: keep pt[p, j] where j >= p (q_local >= k_local)
    trilf = const.tile([P, P], f32)
    nc.vector.memset(trilf[:], 1.0)
    nc.gpsimd.affine_select(
        out=trilf[:], in_=trilf[:], compare_op=mybir.AluOpType.is_ge,
        fill=0.0, base=0, pattern=[[1, P]], channel_multiplier=-1)
    trilb = const.tile([P, P], bf16)
    nc.vector.tensor_copy(trilb[:], trilf[:])

    # ---------------- xT via transpose DMA ----------------
    xT = []  # 8 tiles [128 c, 2048 t] bf16
    for r in range(NCHUNK):
        t_ = xTp.tile([P, T], bf16, name=f"xT{r}", tag="xT")
        xT.append(t_)
    # first t-half of every chunk, then second half: lets qkv for the first
    # t-half (and v for t<8) start while the tail of x is still in flight.
    for half in range(2):
        t0 = half * (T // 2)
        for r in range(NCHUNK):
            nc.sync.dma_start_transpose(
                xT[r][:, t0:t0 + T // 2],
                x_d[t0:t0 + T // 2, r * P:(r + 1) * P],
            )

    # ---------------- qkv / proj emit units ----------------
    qkT = []  # bf16 tiles [128 c', 2048 t]; 0..3 = qT, 4..7 = kT
    for ct in range(8):
        o_t = qkp.tile([P, T], bf16, name=f"qkT{ct}", tag="qkT")
        qkT.append(o_t)

    vaug = []  # [128 k, 8 heads, 65] bf16 per k-chunk (col 64 = ones)
    for t in range(NT):
        va = vap.tile([P, NHL, HD + 1], bf16, name=f"vaug{t}", tag="vaug")
        nc.vector.tensor_copy(va[:, :, HD:HD + 1], ones8[:])
        vaug.append(va)

    wqkv_r = wqkv_d.rearrange("(a p) n -> p a n", p=P)  # [128, 8, 1536]

    wqk_tiles = {}

    def QK(ct, q):
        # one 512-wide quarter of qkT[ct] (t cols q*512..)
        def fn():
            if ct not in wqk_tiles:
                col0 = ct * P
                w_t = wqkp.tile([P, NCHUNK, P], bf16, name=f"wqk_{ct}",
                                tag="wqk")
                nc.sync.dma_start(w_t[:], wqkv_r[:, :, col0:col0 + P])
                wqk_tiles[ct] = w_t
            w_t = wqk_tiles[ct]
            ps = ps_main.tile([P, 1024], f32, name=f"qkps_{ct}_{q}", tag="main")
            for a in range(NCHUNK):
                nc.tensor.matmul(
                    ps[:, 0:512],
                    w_t[:, a, :],
                    xT[a][:, q * 512:(q + 1) * 512],
                    start=(a == 0),
                    stop=(a == NCHUNK - 1),
                )
            nc.vector.tensor_scalar_add(
                qkT[ct][:, q * 512:(q + 1) * 512], ps[:, 0:512],
                bqk_sb[:, ct:ct + 1])
        return fn

    wv_tile = {}

    def V(t):
        def fn():
            if not wv_tile:
                w_t = wvp.tile([P, NCHUNK, GQ], bf16, name="wv", tag="wv")
                nc.sync.dma_start(w_t[:], wqkv_r[:, :, 2 * GQ:3 * GQ])
                wv_tile[0] = w_t
            w_t = wv_tile[0]
            ps = ps_main.tile([P, 1024], f32, name=f"vps_{t}", tag="main")
            for a in range(NCHUNK):
                nc.tensor.matmul(
                    ps[:, 0:512],
                    xT[a][:, t * P:(t + 1) * P],
                    w_t[:, a, :],
                    start=(a == 0),
                    stop=(a == NCHUNK - 1),
                )
            nc.vector.tensor_add(
                vaug[t][:, :, 0:HD],
                ps[:, 0:512].rearrange("p (h d) -> p h d", h=NHL),
                bvb[:].rearrange("p (h d) -> p h d", h=NHL),
            )
        return fn

    wp_tile = {}
    ot_all = {}  # (hp, m) -> [128, 512] bf16 tile in SBUF

    def PJ(m, mt):
        # one cout tile (128 rows of yT) for t window m
        def fn():
            if not wp_tile:
                w_t = wpp.tile([P, 4, C], bf16, name="wp", tag="wp")
                nc.sync.dma_start(w_t[:], wp_d.rearrange("(a p) n -> p a n", p=P))
                wp_tile[0] = w_t
            w_t = wp_tile[0]
            ps = ps_main.tile([P, 1024], f32, name=f"yps_{m}_{mt}", tag="main")
            for a in range(4):
                nc.tensor.matmul(
                    ps[:, 0:512],
                    w_t[:, a, mt * P:(mt + 1) * P],
                    ot_all[(a, m)][:, :],
                    start=(a == 0),
                    stop=(a == 3),
                )
            ys = ysp.tile([P, 512], f32, name=f"ys_{m}_{mt}", tag="ys")
            nc.vector.tensor_copy(ys[:], ps[:, 0:512])
            nc.sync.dma_start(
                yT_d[mt * P:(mt + 1) * P, m * 512:(m + 1) * 512], ys[:])
        return fn

    # ---------------- attention ----------------
    # Head pairs: head A on PE row strip 0, head B on strip 64; score pieces
    # for the two heads live in the two banks of one [128,1024] psum tile, so
    # the row-packed matmuls run concurrently and one exp covers both heads.
    def attn_pair(hp, sched):
        qt = qkT[hp]
        kt = qkT[4 + hp]
        for m in range(4):  # quarter windows of 512 q
            ws = m * 512
            pvt = {}
            for hh in range(2):
                pvt[hh] = ps_pv.tile(
                    [P, 512], f32, name=f"pv_{hp}_{m}_{hh}", tag="ps_pv")
            for i in range(4 * m + 4):  # causal k-chunks for this window
                for fn in sched.get((m, i), ()):
                    fn()
                s = max(i * P, ws)
                o = s - ws
                # head A piece in cols [o, 512), head B in [512, 1024-o)
                sc = ps_main.tile([P, 1024], f32, name=f"sc_{hp}_{m}_{i}",
                                  tag="main")
                for hh in range(2):
                    r0 = hh * HD
                    c0 = o if hh == 0 else 512
                    nc.tensor.matmul(
                        sc[:, c0:c0 + 512 - o],
                        kt[r0:r0 + HD, i * P:(i + 1) * P],
                        qt[r0:r0 + HD, s:ws + 512],
                        start=True,
                        stop=True,
                    )
                pt = ptp.tile([P, 1024], bf16, name=f"pt_{hp}_{m}_{i}",
                              tag="pt")
                nc.scalar.activation(pt[:, o:1024 - o], sc[:, o:1024 - o],
                                     EXP, scale=SCALE)
                diag = i * P >= ws
                for hh in range(2):
                    c0 = o if hh == 0 else 512
                    if diag:
                        nc.vector.tensor_mul(
                            pt[:, c0:c0 + P], pt[:, c0:c0 + P], trilb[:])
                    nc.tensor.matmul(
                        pvt[hh][0:HD + 1, o:],
                        vaug[i][:, 2 * hp + hh, :],
                        pt[:, c0:c0 + 512 - o],
                        start=(i == 0),
                        stop=(i == 4 * m + 3),
                    )
            # normalize both heads: ot rows 0:64 = head A, 64:128 = head B
            ot = otp.tile([P, 512], bf16, name=f"ot_{hp}_{m}", tag="ot",
                          bufs=16)
            for hh in range(2):
                # denom to sbuf partition 0 first: reciprocal_approx_fast
                # mis-handles nonzero partition offsets on HW
                dn = rcp.tile([1, 512], f32, name=f"dn_{hp}_{m}_{hh}", tag="dn")
                nc.vector.tensor_copy(dn[:], pvt[hh][HD:HD + 1, :])
                rc = rcp.tile([1, 512], f32, name=f"rc_{hp}_{m}_{hh}", tag="rc")
                nc.vector.reciprocal_approx_fast(rc[:], dn[:])
                rcb = rcp.tile([HD, 512], f32, name=f"rcb_{hp}_{m}_{hh}",
                               tag="rcb")
                nc.gpsimd.partition_broadcast(rcb[:], rc[:])
                nc.vector.tensor_mul(
                    ot[hh * HD:(hh + 1) * HD, :], pvt[hh][0:HD, :], rcb[:])
            ot_all[(hp, m)] = ot

    # ---------------- schedule ----------------
    # pre-work: just enough for pair 0 window 0
    for t in range(4):
        V(t)()
    QK(0, 0)()
    QK(4, 0)()

    # filler spread at chunk granularity; keys are (window, chunk) with each
    # unit placed before its consumer (deadlines noted inline).
    s0 = {
        (0, 1): [QK(0, 1)],                 # q q1: due p0w1c0
        (1, 0): [V(4)], (1, 1): [V(5)],
        (1, 2): [QK(4, 1), V(6)],           # k q1: due w1c4
        (1, 3): [V(7)], (1, 5): [QK(0, 2)],  # q q2: due w2c0
        (2, 0): [V(8)], (2, 2): [V(9)],
        (2, 4): [QK(4, 2)],                 # k q2: due w2c8
        (2, 6): [V(10)], (2, 8): [V(11), QK(0, 3)],  # q q3: due w3c0
        (3, 0): [V(12)], (3, 2): [V(13)], (3, 4): [V(14)], (3, 6): [V(15)],
        (3, 8): [QK(4, 3)],                 # k q3: due w3c12
        (3, 10): [QK(1, 0)], (3, 12): [QK(5, 0)],   # pair 1 w0
    }
    s1 = {
        (0, 1): [QK(1, 1)],
        (1, 2): [QK(5, 1)],
        (1, 5): [QK(1, 2)],
        (2, 4): [QK(5, 2)],
        (2, 8): [QK(1, 3)],
        (3, 8): [QK(5, 3)],
        (3, 2): [QK(2, 0)], (3, 5): [QK(6, 0)], (3, 10): [QK(2, 1)],
    }
    s2 = {
        (1, 2): [QK(6, 1)],
        (1, 5): [QK(2, 2)],
        (2, 4): [QK(6, 2)],
        (2, 8): [QK(2, 3)],
        (3, 8): [QK(6, 3)],
        (3, 2): [QK(3, 0)], (3, 5): [QK(7, 0)], (3, 10): [QK(3, 1)],
    }
    s3 = {
        (1, 2): [QK(7, 1)],
        (1, 5): [QK(3, 2)],
        (2, 4): [QK(7, 2)],
        (2, 8): [QK(3, 3)],
        (3, 8): [QK(7, 3)],
        # proj for windows 0..2 spread behind their completion
        (1, 0): [PJ(0, 0)], (1, 1): [PJ(0, 1)], (1, 3): [PJ(0, 2)],
        (1, 4): [PJ(0, 3)], (1, 6): [PJ(0, 4)], (1, 7): [PJ(0, 5)],
        (2, 0): [PJ(0, 6)], (2, 1): [PJ(0, 7)],
        (2, 2): [PJ(1, 0)], (2, 3): [PJ(1, 1)], (2, 5): [PJ(1, 2)],
        (2, 6): [PJ(1, 3)], (2, 7): [PJ(1, 4)], (2, 9): [PJ(1, 5)],
        (2, 10): [PJ(1, 6)], (2, 11): [PJ(1, 7)],
        (3, 0): [PJ(2, 0)], (3, 1): [PJ(2, 1)], (3, 3): [PJ(2, 2)],
        (3, 4): [PJ(2, 3)], (3, 6): [PJ(2, 4)], (3, 7): [PJ(2, 5)],
        (3, 9): [PJ(2, 6)], (3, 11): [PJ(2, 7)],
    }
    attn_pair(0, s0)
    attn_pair(1, s1)
    attn_pair(2, s2)
    attn_pair(3, s3)
    for mt in range(NCHUNK):
        PJ(3, mt)()


def _build_program():
    import contextlib

    import concourse.bass as bass
    import concourse.mybir as mybir
    import concourse.tile as tile
    from concourse import bacc

    nc = bacc.Bacc("TRN2", target_bir_lowering=False, debug=False, num_devices=8)
    f32 = mybir.dt.float32
    bf16 = mybir.dt.bfloat16
    aps = {
        "x": nc.dram_tensor("x", [T, C], bf16, kind="ExternalInput").ap(),
        "wqkv": nc.dram_tensor("wqkv", [C, 3 * GQ], bf16, kind="ExternalInput").ap(),
        "bqk": nc.dram_tensor("bqk", [P, 8], f32, kind="ExternalInput").ap(),
        "bv": nc.dram_tensor("bv", [GQ], f32, kind="ExternalInput").ap(),
        "wp": nc.dram_tensor("wp", [GQ, C], bf16, kind="ExternalInput").ap(),
        "yT": nc.dram_tensor("yT", [C, T], f32, kind="ExternalOutput").ap(),
    }
    with tile.TileContext(nc) as tc:
        with contextlib.ExitStack() as ctx:
            _emit(ctx, tc, aps, mybir, bass)
    nc.compile()
    return nc


def get_program():
    global _PROGRAM
    if _PROGRAM is None:
        _PROGRAM = _build_program()
    return _PROGRAM


def make_in_maps(x, w_qkv, b_qkv, w_proj):
    import ml_dtypes

    bf16 = ml_dtypes.bfloat16
    x = np.asarray(x, np.float32)
    w_qkv = np.asarray(w_qkv, np.float32)
    b_qkv = np.asarray(b_qkv, np.float32)
    w_proj = np.asarray(w_proj, np.float32)
    in_maps = []
    for c in range(8):
        b = c // 2
        g = c % 2
        q0 = g * GQ
        wq = w_qkv[:, q0:q0 + GQ]
        wk = w_qkv[:, C + q0:C + q0 + GQ]
        wv = w_qkv[:, 2 * C + q0:2 * C + q0 + GQ]
        wqkv = np.ascontiguousarray(
            np.concatenate([wq, wk, wv], axis=1).astype(bf16))
        bq = b_qkv[q0:q0 + GQ]
        bk = b_qkv[C + q0:C + q0 + GQ]
        bqk = np.ascontiguousarray(np.concatenate([bq, bk]).reshape(8, P).T)
        bv = np.ascontiguousarray(b_qkv[2 * C + q0:2 * C + q0 + GQ])
        in_maps.append({
            "x": np.ascontiguousarray(x[b].astype(bf16)),
            "wqkv": wqkv,
            "bqk": bqk,
            "bv": bv,
            "wp": np.ascontiguousarray(w_proj[q0:q0 + GQ, :].astype(bf16)),
        })
    return in_maps


def combine_outputs(outs, b_proj):
    b_proj = np.asarray(b_proj, np.float32)
    y = np.empty((B, T, C), np.float32)
    for b in range(B):
        acc = outs[2 * b] + outs[2 * b + 1]  # [C, T]
        y[b] = acc.T + b_proj
    return y


def kernel(x, w_qkv, b_qkv, w_proj, b_proj, _trace=False):
    from concourse import bass_utils

    nc = get_program()
    in_maps = make_in_maps(x, w_qkv, b_qkv, w_proj)
    res = bass_utils.run_bass_kernel_spmd(
        nc, in_maps, core_ids=list(range(8)), trace=_trace
    )
    outs = [r["yT"] for r in res.results]
    y = combine_outputs(outs, b_proj)
    if _trace:
        return y, res
    return y


# revision 6
# speedup vs baseline: 1.1844x; 1.0846x over previous
"""Causal self-attention on 8 TRN2 NeuronCores.

Sharding: core c handles batch b = c//2 and head-group g = c%2 (8 of 16 heads).
Each core computes its partial y^T = w_proj[slice].T @ o^T (contraction over its
512 o-channels); the host sums the two partials per batch and adds b_proj.

Shapes (hardcoded): B=4, T=2048, C=1024, H=16, HD=64.

All matmul operands are bf16 (x/w_qkv/w_proj cast on host); accumulation is
fp32 in PSUM. x^T is loaded straight from DRAM with the xbar transpose DMA
(issues split across the SP and ACT queues; weight DMAs issued first).
o stays in SBUF (bf16) and feeds proj directly.

Schedule: attention is ACT(exp)-bound, so qkv/v/proj work is emitted in
half-unit (4-matmul) chunks interleaved between attention chunks, keeping the
PE stream dense while ACT crunches exp without starving its 2-deep score
backlog. proj for window m runs inside pair 3 right after (3, m) completes.
Diagonal causal masking is a DVE multiply with a tril mask (gpsimd
affine_select is broken for bf16 on HW, and gpsimd cannot read PSUM).

PSUM (8 banks): ps_main 2x[128,1024] holds score tiles AND filler accumulators
(split filler halves interleave 1:1 with score allocs so rotation deps always
point backward); ps_pv 2x[128,1024] holds the per-window PV accumulator — both
heads side by side, so one reciprocal-normalize chain covers the window.
reciprocal_approx_fast needs its input at partition offset 0 (HW bug), hence
the denominator row is first copied to a [1,1024] sbuf tile.
"""

import numpy as np

B, T, C, H = 4, 2048, 1024, 16
HD = C // H          # 64
G = 2                # head groups
NHL = H // G         # 8 heads per core
GQ = NHL * HD        # 512 channel slice per core
P = 128
NT = T // P          # 16 token tiles / k-chunks
NCHUNK = C // P      # 8 contraction chunks for qkv
SCALE = 1.0 / float(np.sqrt(HD))

_PROGRAM = None


def _emit(ctx, tc, aps, mybir, bass):
    nc = tc.nc
    f32 = mybir.dt.float32
    bf16 = mybir.dt.bfloat16
    EXP = mybir.ActivationFunctionType.Exp

    x_d, wqkv_d, bqk_d, bv_d, wp_d, yT_d = (
        aps["x"], aps["wqkv"], aps["bqk"], aps["bv"], aps["wp"], aps["yT"],
    )

    # ---------------- pools ----------------
    const = ctx.enter_context(tc.tile_pool(name="const", bufs=1))
    ps_main = ctx.enter_context(tc.tile_pool(name="ps_main", bufs=2, space="PSUM"))
    ps_pv = ctx.enter_context(tc.tile_pool(name="ps_pv", bufs=2, space="PSUM"))

    qkp = ctx.enter_context(tc.tile_pool(name="qkp", bufs=8))
    vap = ctx.enter_context(tc.tile_pool(name="vap", bufs=16))
    ptp = ctx.enter_context(tc.tile_pool(name="ptp", bufs=3))
    otp = ctx.enter_context(tc.tile_pool(name="otp", bufs=16))
    rcp = ctx.enter_context(tc.tile_pool(name="rcp", bufs=2))
    xTp = ctx.enter_context(tc.tile_pool(name="xTp", bufs=8))
    wqkp = ctx.enter_context(tc.tile_pool(name="wqkp", bufs=4))
    wvp = ctx.enter_context(tc.tile_pool(name="wvp", bufs=1))
    wpp = ctx.enter_context(tc.tile_pool(name="wpp", bufs=1))
    ysp = ctx.enter_context(tc.tile_pool(name="ysp", bufs=3))

    # constants
    bqk_sb = const.tile([P, 8], f32)
    nc.sync.dma_start(bqk_sb[:], bqk_d[:])
    bvb = const.tile([P, GQ], f32)
    nc.sync.dma_start(bvb[:], bv_d[None, :].to_broadcast((P, GQ)))
    ones8 = const.tile([P, NHL, 1], f32)
    nc.vector.memset(ones8[:], 1.0)
    # tril causal mask, bf16: keep pt[p, j] where j >= p (q_local >= k_local)
    trilf = const.tile([P, P], f32)
    nc.vector.memset(trilf[:], 1.0)
    nc.gpsimd.affine_select(
        out=trilf[:], in_=trilf[:], compare_op=mybir.AluOpType.is_ge,
        fill=0.0, base=0, pattern=[[1, P]], channel_multiplier=-1)
    trilb = const.tile([P, P], bf16)
    nc.vector.tensor_copy(trilb[:], trilf[:])

    wqkv_r = wqkv_d.rearrange("(a p) n -> p a n", p=P)  # [128, 8, 1536]

    # ---------------- weight DMAs first (small, unblock qkv) ------------
    wqk_tiles = {}

    def load_wqk(ct):
        w_t = wqkp.tile([P, NCHUNK, P], bf16, name=f"wqk_{ct}", tag="wqk")
        nc.sync.dma_start(w_t[:], wqkv_r[:, :, ct * P:ct * P + P])
        wqk_tiles[ct] = w_t

    load_wqk(0)
    load_wqk(4)
    wv_t = wvp.tile([P, NCHUNK, GQ], bf16, name="wv", tag="wv")
    nc.sync.dma_start(wv_t[:], wqkv_r[:, :, 2 * GQ:3 * GQ])
    wp_t = wpp.tile([P, 4, C], bf16, name="wp", tag="wp")
    nc.sync.dma_start(wp_t[:], wp_d.rearrange("(a p) n -> p a n", p=P))

    # ---------------- xT via transpose DMA ----------------
    xT = []  # 8 tiles [128 c, 2048 t] bf16
    for r in range(NCHUNK):
        t_ = xTp.tile([P, T], bf16, name=f"xT{r}", tag="xT")
        xT.append(t_)
    # first t-half of every chunk, then second half (all on the SP queue:
    # ACT-issued transpose DMAs corrupt data on HW).
    for half in range(2):
        t0 = half * (T // 2)
        for r in range(NCHUNK):
            nc.sync.dma_start_transpose(
                xT[r][:, t0:t0 + T // 2],
                x_d[t0:t0 + T // 2, r * P:(r + 1) * P],
            )

    # ---------------- qkv / proj emit units ----------------
    qkT = []  # bf16 tiles [128 c', 2048 t]; 0..3 = qT, 4..7 = kT
    for ct in range(8):
        o_t = qkp.tile([P, T], bf16, name=f"qkT{ct}", tag="qkT")
        qkT.append(o_t)

    vaug = []  # [128 k, 8 heads, 65] bf16 per k-chunk (col 64 = ones)
    for t in range(NT):
        va = vap.tile([P, NHL, HD + 1], bf16, name=f"vaug{t}", tag="vaug")
        nc.vector.tensor_copy(va[:, :, HD:HD + 1], ones8[:])
        vaug.append(va)

    def QK(ct, q):
        # one 512-wide quarter of qkT[ct], split into two 4-contraction halves
        st = {}

        def fn1():
            if ct not in wqk_tiles:
                load_wqk(ct)
            ps = ps_main.tile([P, 1024], f32, name=f"qkps_{ct}_{q}", tag="main")
            st["ps"] = ps
            for a in range(4):
                nc.tensor.matmul(
                    ps[:, 0:512], wqk_tiles[ct][:, a, :],
                    xT[a][:, q * 512:(q + 1) * 512],
                    start=(a == 0), stop=False)

        def fn2():
            ps = st["ps"]
            for a in range(4, NCHUNK):
                nc.tensor.matmul(
                    ps[:, 0:512], wqk_tiles[ct][:, a, :],
                    xT[a][:, q * 512:(q + 1) * 512],
                    start=False, stop=(a == NCHUNK - 1))
            nc.vector.tensor_scalar_add(
                qkT[ct][:, q * 512:(q + 1) * 512], ps[:, 0:512],
                bqk_sb[:, ct:ct + 1])
        return fn1, fn2

    def V(t):
        st = {}

        def fn1():
            ps = ps_main.tile([P, 1024], f32, name=f"vps_{t}", tag="main")
            st["ps"] = ps
            for a in range(4):
                nc.tensor.matmul(
                    ps[:, 0:512], xT[a][:, t * P:(t + 1) * P], wv_t[:, a, :],
                    start=(a == 0), stop=False)

        def fn2():
            ps = st["ps"]
            for a in range(4, NCHUNK):
                nc.tensor.matmul(
                    ps[:, 0:512], xT[a][:, t * P:(t + 1) * P], wv_t[:, a, :],
                    start=False, stop=(a == NCHUNK - 1))
            nc.vector.tensor_add(
                vaug[t][:, :, 0:HD],
                ps[:, 0:512].rearrange("p (h d) -> p h d", h=NHL),
                bvb[:].rearrange("p (h d) -> p h d", h=NHL))
        return fn1, fn2

    ot_all = {}  # (hp, m) -> [128, 512] bf16 tile in SBUF

    def PJ(m, mt):
        # one cout tile (128 rows of yT) for t window m; atomic (4 matmuls)
        def fn():
            ps = ps_main.tile([P, 1024], f32, name=f"yps_{m}_{mt}", tag="main")
            for a in range(4):
                nc.tensor.matmul(
                    ps[:, 0:512], wp_t[:, a, mt * P:(mt + 1) * P],
                    ot_all[(a, m)][:, :],
                    start=(a == 0), stop=(a == 3))
            ys = ysp.tile([P, 512], f32, name=f"ys_{m}_{mt}", tag="ys")
            nc.scalar.copy(ys[:], ps[:, 0:512])
            nc.sync.dma_start(
                yT_d[mt * P:(mt + 1) * P, m * 512:(m + 1) * 512], ys[:])
        return fn

    # ---------------- attention ----------------
    # Head pairs: head A on PE row strip 0, head B on strip 64; score pieces
    # for the two heads live in the two banks of one [128,1024] psum tile, so
    # the row-packed matmuls run concurrently and one exp covers both heads.
    # The PV accumulator is likewise one [128,1024] tile: head A cols 0:512,
    # head B cols 512:1024, partition 64 = denominators (ones column of vaug).
    def attn_pair(hp, sched):
        qt = qkT[hp]
        kt = qkT[4 + hp]
        for m in range(4):  # quarter windows of 512 q
            ws = m * 512
            pvt = ps_pv.tile([P, 1024], f32, name=f"pv_{hp}_{m}", tag="ps_pv")
            for i in range(4 * m + 4):  # causal k-chunks for this window
                for fn in sched.get((m, i), ()):
                    fn()
                s = max(i * P, ws)
                o = s - ws
                # head A piece in cols [o, 512), head B in [512, 1024-o)
                sc = ps_main.tile([P, 1024], f32, name=f"sc_{hp}_{m}_{i}",
                                  tag="main")
                for hh in range(2):
                    r0 = hh * HD
                    c0 = o if hh == 0 else 512
                    nc.tensor.matmul(
                        sc[:, c0:c0 + 512 - o],
                        kt[r0:r0 + HD, i * P:(i + 1) * P],
                        qt[r0:r0 + HD, s:ws + 512],
                        start=True,
                        stop=True,
                    )
                pt = ptp.tile([P, 1024], bf16, name=f"pt_{hp}_{m}_{i}",
                              tag="pt")
                nc.scalar.activation(pt[:, o:1024 - o], sc[:, o:1024 - o],
                                     EXP, scale=SCALE)
                diag = i * P >= ws
                for hh in range(2):
                    c0 = o if hh == 0 else 512
                    if diag:
                        nc.vector.tensor_mul(
                            pt[:, c0:c0 + P], pt[:, c0:c0 + P], trilb[:])
                    nc.tensor.matmul(
                        pvt[0:HD + 1, hh * 512 + o:(hh + 1) * 512],
                        vaug[i][:, 2 * hp + hh, :],
                        pt[:, c0:c0 + 512 - o],
                        start=(i == 0),
                        stop=(i == 4 * m + 3),
                    )
            # normalize both heads at once: denominators to sbuf partition 0
            # (reciprocal_approx_fast mishandles nonzero partition offsets)
            dn = rcp.tile([1, 1024], f32, name=f"dn_{hp}_{m}", tag="dn")
            nc.vector.tensor_copy(dn[:], pvt[HD:HD + 1, :])
            rc = rcp.tile([1, 1024], f32, name=f"rc_{hp}_{m}", tag="rc")
            nc.vector.reciprocal_approx_fast(rc[:], dn[:])
            rcb = rcp.tile([HD, 1024], f32, name=f"rcb_{hp}_{m}", tag="rcb")
            nc.gpsimd.partition_broadcast(rcb[:], rc[:])
            ot = otp.tile([P, 512], bf16, name=f"ot_{hp}_{m}", tag="ot",
                          bufs=16)
            for hh in range(2):
                nc.vector.tensor_mul(
                    ot[hh * HD:(hh + 1) * HD, :],
                    pvt[0:HD, hh * 512:(hh + 1) * 512],
                    rcb[:, hh * 512:(hh + 1) * 512])
            ot_all[(hp, m)] = ot

    # ---------------- schedule ----------------
    # pre-work: enough for pair 0 window 0 plus v tiles 4,5 (w1 is packed)
    for t in range(6):
        f1, f2 = V(t)
        f1()
        f2()
    for ct in (0, 4):
        f1, f2 = QK(ct, 0)
        f1()
        f2()

    def mk():
        return {}

    def put2(s, m, i, unit):
        # split unit: half 1 before chunk (m,i), half 2 before chunk (m,i+1)
        f1, f2 = unit
        s.setdefault((m, i), []).append(f1)
        s.setdefault((m, i + 1), []).append(f2)

    def put1(s, m, i, fn):
        s.setdefault((m, i), []).append(fn)

    # pair 0: carries all remaining V units + its own q/k quarters + pair 1 q0/k0
    s0 = mk()
    put2(s0, 0, 0, QK(0, 1))
    put2(s0, 1, 0, QK(4, 1))   # due w1c4
    put2(s0, 1, 2, V(6))       # due w1c6
    put2(s0, 1, 4, V(7))       # due w1c7
    put2(s0, 1, 6, QK(0, 2))   # due w2c0
    put2(s0, 2, 0, QK(4, 2))   # due w2c8
    put2(s0, 2, 2, V(8))
    put2(s0, 2, 4, V(9))
    put2(s0, 2, 6, V(10))
    put2(s0, 2, 8, V(11))
    put2(s0, 2, 10, QK(0, 3))  # due w3c0
    put2(s0, 3, 0, QK(4, 3))   # due w3c12
    put2(s0, 3, 2, V(12))
    put2(s0, 3, 4, V(13))
    put2(s0, 3, 6, V(14))
    put2(s0, 3, 8, V(15))
    put2(s0, 3, 10, QK(1, 0))  # pair 1 w0
    put2(s0, 3, 12, QK(5, 0))

    s1 = mk()
    put2(s1, 0, 0, QK(1, 1))
    put2(s1, 1, 0, QK(5, 1))
    put2(s1, 1, 4, QK(1, 2))
    put2(s1, 2, 0, QK(5, 2))
    put2(s1, 2, 4, QK(1, 3))
    put2(s1, 2, 8, QK(2, 0))
    put2(s1, 3, 0, QK(5, 3))
    put2(s1, 3, 2, QK(6, 0))
    put2(s1, 3, 4, QK(2, 1))

    s2 = mk()
    put2(s2, 1, 0, QK(6, 1))
    put2(s2, 1, 4, QK(2, 2))
    put2(s2, 2, 0, QK(6, 2))
    put2(s2, 2, 4, QK(2, 3))
    put2(s2, 2, 8, QK(3, 0))
    put2(s2, 3, 0, QK(6, 3))
    put2(s2, 3, 2, QK(7, 0))
    put2(s2, 3, 4, QK(3, 1))

    s3 = mk()
    put2(s3, 1, 0, QK(7, 1))
    put2(s3, 1, 4, QK(3, 2))
    put1(s3, 1, 6, PJ(0, 0))
    put1(s3, 1, 7, PJ(0, 1))
    put2(s3, 2, 0, QK(7, 2))
    put1(s3, 2, 2, PJ(0, 2))
    put1(s3, 2, 3, PJ(0, 3))
    put2(s3, 2, 4, QK(3, 3))
    put1(s3, 2, 6, PJ(0, 4))
    put1(s3, 2, 7, PJ(0, 5))
    put1(s3, 2, 8, PJ(0, 6))
    put1(s3, 2, 9, PJ(0, 7))
    put1(s3, 2, 10, PJ(1, 0))
    put1(s3, 2, 11, PJ(1, 1))
    put2(s3, 3, 0, QK(7, 3))
    for k in range(6):
        put1(s3, 3, 2 + k, PJ(1, 2 + k))
    for k in range(8):
        put1(s3, 3, 8 + k, PJ(2, k))

    attn_pair(0, s0)
    attn_pair(1, s1)
    attn_pair(2, s2)
    attn_pair(3, s3)
    for mt in range(NCHUNK):
        PJ(3, mt)()


def _build_program():
    import contextlib

    import concourse.bass as bass
    import concourse.mybir as mybir
    import concourse.tile as tile
    from concourse import bacc

    nc = bacc.Bacc("TRN2", target_bir_lowering=False, debug=False, num_devices=8)
    f32 = mybir.dt.float32
    bf16 = mybir.dt.bfloat16
    aps = {
        "x": nc.dram_tensor("x", [T, C], bf16, kind="ExternalInput").ap(),
        "wqkv": nc.dram_tensor("wqkv", [C, 3 * GQ], bf16, kind="ExternalInput").ap(),
        "bqk": nc.dram_tensor("bqk", [P, 8], f32, kind="ExternalInput").ap(),
        "bv": nc.dram_tensor("bv", [GQ], f32, kind="ExternalInput").ap(),
        "wp": nc.dram_tensor("wp", [GQ, C], bf16, kind="ExternalInput").ap(),
        "yT": nc.dram_tensor("yT", [C, T], f32, kind="ExternalOutput").ap(),
    }
    with tile.TileContext(nc) as tc:
        with contextlib.ExitStack() as ctx:
            _emit(ctx, tc, aps, mybir, bass)
    nc.compile()
    return nc


def get_program():
    global _PROGRAM
    if _PROGRAM is None:
        _PROGRAM = _build_program()
    return _PROGRAM


def make_in_maps(x, w_qkv, b_qkv, w_proj):
    import ml_dtypes

    bf16 = ml_dtypes.bfloat16
    x = np.asarray(x, np.float32)
    w_qkv = np.asarray(w_qkv, np.float32)
    b_qkv = np.asarray(b_qkv, np.float32)
    w_proj = np.asarray(w_proj, np.float32)
    in_maps = []
    for c in range(8):
        b = c // 2
        g = c % 2
        q0 = g * GQ
        wq = w_qkv[:, q0:q0 + GQ]
        wk = w_qkv[:, C + q0:C + q0 + GQ]
        wv = w_qkv[:, 2 * C + q0:2 * C + q0 + GQ]
        wqkv = np.ascontiguousarray(
            np.concatenate([wq, wk, wv], axis=1).astype(bf16))
        bq = b_qkv[q0:q0 + GQ]
        bk = b_qkv[C + q0:C + q0 + GQ]
        bqk = np.ascontiguousarray(np.concatenate([bq, bk]).reshape(8, P).T)
        bv = np.ascontiguousarray(b_qkv[2 * C + q0:2 * C + q0 + GQ])
        in_maps.append({
            "x": np.ascontiguousarray(x[b].astype(bf16)),
            "wqkv": wqkv,
            "bqk": bqk,
            "bv": bv,
            "wp": np.ascontiguousarray(w_proj[q0:q0 + GQ, :].astype(bf16)),
        })
    return in_maps


def combine_outputs(outs, b_proj):
    b_proj = np.asarray(b_proj, np.float32)
    y = np.empty((B, T, C), np.float32)
    for b in range(B):
        acc = outs[2 * b] + outs[2 * b + 1]  # [C, T]
        y[b] = acc.T + b_proj
    return y


def kernel(x, w_qkv, b_qkv, w_proj, b_proj, _trace=False):
    from concourse import bass_utils

    nc = get_program()
    in_maps = make_in_maps(x, w_qkv, b_qkv, w_proj)
    res = bass_utils.run_bass_kernel_spmd(
        nc, in_maps, core_ids=list(range(8)), trace=_trace
    )
    outs = [r["yT"] for r in res.results]
    y = combine_outputs(outs, b_proj)
    if _trace:
        return y, res
    return y


# revision 9
# speedup vs baseline: 1.1973x; 1.0109x over previous
"""Causal self-attention on 8 TRN2 NeuronCores.

Sharding: core c handles batch b = c//2 and head-group g = c%2 (8 of 16 heads).
Each core computes its partial y^T = w_proj[slice].T @ o^T (contraction over its
512 o-channels); the host sums the two partials per batch and adds b_proj.

Shapes (hardcoded): B=4, T=2048, C=1024, H=16, HD=64.

All matmul operands are bf16 (x/w_qkv/w_proj cast on host); accumulation is
fp32 in PSUM. x^T is loaded straight from DRAM with the xbar transpose DMA
(issues split across the SP and ACT queues; weight DMAs issued first).
o stays in SBUF (bf16) and feeds proj directly.

Schedule: attention is ACT(exp)-bound, so qkv/v/proj work is emitted in
half-unit (4-matmul) chunks interleaved between attention chunks, keeping the
PE stream dense while ACT crunches exp without starving its 2-deep score
backlog. proj for window m runs inside pair 3 right after (3, m) completes.
Diagonal causal masking is a DVE multiply with a tril mask (gpsimd
affine_select is broken for bf16 on HW, and gpsimd cannot read PSUM).

PSUM (8 banks): ps_main 2x[128,1024] holds score tiles AND filler accumulators
(split filler halves interleave 1:1 with score allocs so rotation deps always
point backward); ps_pv 2x[128,1024] holds the per-window PV accumulator — both
heads side by side, so one reciprocal-normalize chain covers the window.
reciprocal_approx_fast needs its input at partition offset 0 (HW bug), hence
the denominator row is first copied to a [1,1024] sbuf tile.
"""

import numpy as np

B, T, C, H = 4, 2048, 1024, 16
HD = C // H          # 64
G = 2                # head groups
NHL = H // G         # 8 heads per core
GQ = NHL * HD        # 512 channel slice per core
P = 128
NT = T // P          # 16 token tiles / k-chunks
NCHUNK = C // P      # 8 contraction chunks for qkv
SCALE = 1.0 / float(np.sqrt(HD))

_PROGRAM = None


def _emit(ctx, tc, aps, mybir, bass):
    nc = tc.nc
    f32 = mybir.dt.float32
    bf16 = mybir.dt.bfloat16
    EXP = mybir.ActivationFunctionType.Exp

    x_d, wqkv_d, bqk_d, bv_d, wp_d, yT_d = (
        aps["x"], aps["wqkv"], aps["bqk"], aps["bv"], aps["wp"], aps["yT"],
    )

    # ---------------- pools ----------------
    const = ctx.enter_context(tc.tile_pool(name="const", bufs=1))
    ps_main = ctx.enter_context(tc.tile_pool(name="ps_main", bufs=2, space="PSUM"))
    ps_pv = ctx.enter_context(tc.tile_pool(name="ps_pv", bufs=2, space="PSUM"))

    qkp = ctx.enter_context(tc.tile_pool(name="qkp", bufs=8))
    vap = ctx.enter_context(tc.tile_pool(name="vap", bufs=16))
    ptp = ctx.enter_context(tc.tile_pool(name="ptp", bufs=3))
    otp = ctx.enter_context(tc.tile_pool(name="otp", bufs=16))
    rcp = ctx.enter_context(tc.tile_pool(name="rcp", bufs=2))
    xTp = ctx.enter_context(tc.tile_pool(name="xTp", bufs=8))
    wqkp = ctx.enter_context(tc.tile_pool(name="wqkp", bufs=4))
    wvp = ctx.enter_context(tc.tile_pool(name="wvp", bufs=1))
    wpp = ctx.enter_context(tc.tile_pool(name="wpp", bufs=1))
    ysp = ctx.enter_context(tc.tile_pool(name="ysp", bufs=3))

    # constants
    bqk_sb = const.tile([P, 8], f32)
    nc.sync.dma_start(bqk_sb[:], bqk_d[:])
    bvb = const.tile([P, GQ], f32)
    nc.sync.dma_start(bvb[:], bv_d[None, :].to_broadcast((P, GQ)))
    ones8 = const.tile([P, NHL, 1], f32)
    nc.vector.memset(ones8[:], 1.0)
    # tril causal mask, bf16: keep pt[p, j] where j >= p (q_local >= k_local)
    trilf = const.tile([P, P], f32)
    nc.vector.memset(trilf[:], 1.0)
    nc.gpsimd.affine_select(
        out=trilf[:], in_=trilf[:], compare_op=mybir.AluOpType.is_ge,
        fill=0.0, base=0, pattern=[[1, P]], channel_multiplier=-1)
    trilb = const.tile([P, P], bf16)
    nc.vector.tensor_copy(trilb[:], trilf[:])

    wqkv_r = wqkv_d.rearrange("(a p) n -> p a n", p=P)  # [128, 8, 1536]

    # ---------------- weight DMAs first (small, unblock qkv) ------------
    wqk_tiles = {}

    def load_wqk(ct):
        w_t = wqkp.tile([P, NCHUNK, P], bf16, name=f"wqk_{ct}", tag="wqk")
        nc.sync.dma_start(w_t[:], wqkv_r[:, :, ct * P:ct * P + P])
        wqk_tiles[ct] = w_t

    load_wqk(0)
    load_wqk(4)
    wv_t = wvp.tile([P, NCHUNK, GQ], bf16, name="wv", tag="wv")
    nc.sync.dma_start(wv_t[:], wqkv_r[:, :, 2 * GQ:3 * GQ])
    wp_t = wpp.tile([P, 4, C], bf16, name="wp", tag="wp")
    nc.sync.dma_start(wp_t[:], wp_d.rearrange("(a p) n -> p a n", p=P))

    # ---------------- xT via transpose DMA ----------------
    xT = []  # 8 tiles [128 c, 2048 t] bf16
    for r in range(NCHUNK):
        t_ = xTp.tile([P, T], bf16, name=f"xT{r}", tag="xT")
        xT.append(t_)
    # three batches: t 0:512 (unblocks pair-0 window 0 + V(0..3) fast),
    # t 512:1024, then t 1024:2048. All on the SP queue: ACT-issued
    # transpose DMAs corrupt data on HW.
    for t0, t1 in ((0, 512), (512, 1024), (1024, 2048)):
        for r in range(NCHUNK):
            nc.sync.dma_start_transpose(
                xT[r][:, t0:t1],
                x_d[t0:t1, r * P:(r + 1) * P],
            )

    # ---------------- qkv / proj emit units ----------------
    qkT = []  # bf16 tiles [128 c', 2048 t]; 0..3 = qT, 4..7 = kT
    for ct in range(8):
        o_t = qkp.tile([P, T], bf16, name=f"qkT{ct}", tag="qkT")
        qkT.append(o_t)

    vaug = []  # [128 k, 8 heads, 65] bf16 per k-chunk (col 64 = ones)
    for t in range(NT):
        va = vap.tile([P, NHL, HD + 1], bf16, name=f"vaug{t}", tag="vaug")
        nc.vector.tensor_copy(va[:, :, HD:HD + 1], ones8[:])
        vaug.append(va)

    def QK(ct, q):
        # one 512-wide quarter of qkT[ct], split into two 4-contraction halves
        st = {}

        def fn1():
            if ct not in wqk_tiles:
                load_wqk(ct)
            ps = ps_main.tile([P, 1024], f32, name=f"qkps_{ct}_{q}", tag="main")
            st["ps"] = ps
            for a in range(4):
                nc.tensor.matmul(
                    ps[:, 0:512], wqk_tiles[ct][:, a, :],
                    xT[a][:, q * 512:(q + 1) * 512],
                    start=(a == 0), stop=False)

        def fn2():
            ps = st["ps"]
            for a in range(4, NCHUNK):
                nc.tensor.matmul(
                    ps[:, 0:512], wqk_tiles[ct][:, a, :],
                    xT[a][:, q * 512:(q + 1) * 512],
                    start=False, stop=(a == NCHUNK - 1))
            nc.vector.tensor_scalar_add(
                qkT[ct][:, q * 512:(q + 1) * 512], ps[:, 0:512],
                bqk_sb[:, ct:ct + 1])
        return fn1, fn2

    def V(t):
        st = {}

        def fn1():
            ps = ps_main.tile([P, 1024], f32, name=f"vps_{t}", tag="main")
            st["ps"] = ps
            for a in range(4):
                nc.tensor.matmul(
                    ps[:, 0:512], xT[a][:, t * P:(t + 1) * P], wv_t[:, a, :],
                    start=(a == 0), stop=False)

        def fn2():
            ps = st["ps"]
            for a in range(4, NCHUNK):
                nc.tensor.matmul(
                    ps[:, 0:512], xT[a][:, t * P:(t + 1) * P], wv_t[:, a, :],
                    start=False, stop=(a == NCHUNK - 1))
            nc.vector.tensor_add(
                vaug[t][:, :, 0:HD],
                ps[:, 0:512].rearrange("p (h d) -> p h d", h=NHL),
                bvb[:].rearrange("p (h d) -> p h d", h=NHL))
        return fn1, fn2

    ot_all = {}  # (hp, m) -> [128, 512] bf16 tile in SBUF

    def PJ(m, mt):
        # one cout tile (128 rows of yT) for t window m; atomic (4 matmuls)
        def fn():
            ps = ps_main.tile([P, 1024], f32, name=f"yps_{m}_{mt}", tag="main")
            for a in range(4):
                nc.tensor.matmul(
                    ps[:, 0:512], wp_t[:, a, mt * P:(mt + 1) * P],
                    ot_all[(a, m)][:, :],
                    start=(a == 0), stop=(a == 3))
            ys = ysp.tile([P, 512], f32, name=f"ys_{m}_{mt}", tag="ys")
            nc.vector.tensor_copy(ys[:], ps[:, 0:512])
            nc.sync.dma_start(
                yT_d[mt * P:(mt + 1) * P, m * 512:(m + 1) * 512], ys[:])
        return fn

    # ---------------- attention ----------------
    # Head pairs: head A on PE row strip 0, head B on strip 64; score pieces
    # for the two heads live in the two banks of one [128,1024] psum tile, so
    # the row-packed matmuls run concurrently and one exp covers both heads.
    # The PV accumulator is likewise one [128,1024] tile: head A cols 0:512,
    # head B cols 512:1024, partition 64 = denominators (ones column of vaug).
    def attn_pair(hp, sched):
        qt = qkT[hp]
        kt = qkT[4 + hp]
        for m in range(4):  # quarter windows of 512 q
            ws = m * 512
            pvt = ps_pv.tile([P, 1024], f32, name=f"pv_{hp}_{m}", tag="ps_pv")
            for i in range(4 * m + 4):  # causal k-chunks for this window
                for fn in sched.get((m, i), ()):
                    fn()
                s = max(i * P, ws)
                o = s - ws
                # head A piece in cols [o, 512), head B in [512, 1024-o)
                sc = ps_main.tile([P, 1024], f32, name=f"sc_{hp}_{m}_{i}",
                                  tag="main")
                for hh in range(2):
                    r0 = hh * HD
                    c0 = o if hh == 0 else 512
                    nc.tensor.matmul(
                        sc[:, c0:c0 + 512 - o],
                        kt[r0:r0 + HD, i * P:(i + 1) * P],
                        qt[r0:r0 + HD, s:ws + 512],
                        start=True,
                        stop=True,
                    )
                pt = ptp.tile([P, 1024], bf16, name=f"pt_{hp}_{m}_{i}",
                              tag="pt")
                nc.scalar.activation(pt[:, o:1024 - o], sc[:, o:1024 - o],
                                     EXP, scale=SCALE)
                diag = i * P >= ws
                for hh in range(2):
                    c0 = o if hh == 0 else 512
                    if diag:
                        nc.vector.tensor_mul(
                            pt[:, c0:c0 + P], pt[:, c0:c0 + P], trilb[:])
                    nc.tensor.matmul(
                        pvt[0:HD + 1, hh * 512 + o:(hh + 1) * 512],
                        vaug[i][:, 2 * hp + hh, :],
                        pt[:, c0:c0 + 512 - o],
                        start=(i == 0),
                        stop=(i == 4 * m + 3),
                    )
            # normalize both heads at once: denominators to sbuf partition 0
            # (reciprocal_approx_fast mishandles nonzero partition offsets)
            dn = rcp.tile([1, 1024], f32, name=f"dn_{hp}_{m}", tag="dn")
            nc.vector.tensor_copy(dn[:], pvt[HD:HD + 1, :])
            rc = rcp.tile([1, 1024], f32, name=f"rc_{hp}_{m}", tag="rc")
            nc.vector.reciprocal_approx_fast(rc[:], dn[:])
            rcb = rcp.tile([HD, 1024], f32, name=f"rcb_{hp}_{m}", tag="rcb")
            nc.gpsimd.partition_broadcast(rcb[:], rc[:])
            ot = otp.tile([P, 512], bf16, name=f"ot_{hp}_{m}", tag="ot",
                          bufs=16)
            for hh in range(2):
                nc.vector.tensor_mul(
                    ot[hh * HD:(hh + 1) * HD, :],
                    pvt[0:HD, hh * 512:(hh + 1) * 512],
                    rcb[:, hh * 512:(hh + 1) * 512])
            ot_all[(hp, m)] = ot

    # ---------------- schedule ----------------
    # pre-work: enough for pair 0 window 0 plus v tiles 4,5 (w1 is packed)
    for t in range(6):
        f1, f2 = V(t)
        f1()
        f2()
    for ct in (0, 4):
        f1, f2 = QK(ct, 0)
        f1()
        f2()

    def mk():
        return {}

    def put2(s, m, i, unit):
        # split unit: half 1 before chunk (m,i), half 2 before chunk (m,i+1)
        f1, f2 = unit
        s.setdefault((m, i), []).append(f1)
        s.setdefault((m, i + 1), []).append(f2)

    def put1(s, m, i, fn):
        s.setdefault((m, i), []).append(fn)

    # pair 0: carries all remaining V units + its own q/k quarters + pair 1 q0/k0
    s0 = mk()
    put2(s0, 0, 0, QK(0, 1))
    put2(s0, 1, 0, QK(4, 1))   # due w1c4
    put2(s0, 1, 2, V(6))       # due w1c6
    put2(s0, 1, 4, V(7))       # due w1c7
    put2(s0, 1, 6, QK(0, 2))   # due w2c0
    put2(s0, 2, 0, QK(4, 2))   # due w2c8
    put2(s0, 2, 2, V(8))
    put2(s0, 2, 4, V(9))
    put2(s0, 2, 6, V(10))
    put2(s0, 2, 8, V(11))
    put2(s0, 2, 10, QK(0, 3))  # due w3c0
    put2(s0, 3, 0, QK(4, 3))   # due w3c12
    put2(s0, 3, 2, V(12))
    put2(s0, 3, 4, V(13))
    put2(s0, 3, 6, V(14))
    put2(s0, 3, 8, V(15))
    put2(s0, 3, 10, QK(1, 0))  # pair 1 w0
    put2(s0, 3, 12, QK(5, 0))

    s1 = mk()
    put2(s1, 0, 0, QK(1, 1))
    put2(s1, 1, 0, QK(5, 1))
    put2(s1, 1, 4, QK(1, 2))
    put2(s1, 2, 0, QK(5, 2))
    put2(s1, 2, 4, QK(1, 3))
    put2(s1, 2, 8, QK(2, 0))
    put2(s1, 3, 0, QK(5, 3))
    put2(s1, 3, 2, QK(6, 0))
    put2(s1, 3, 4, QK(2, 1))

    s2 = mk()
    put2(s2, 1, 0, QK(6, 1))
    put2(s2, 1, 4, QK(2, 2))
    put2(s2, 2, 0, QK(6, 2))
    put2(s2, 2, 4, QK(2, 3))
    put2(s2, 2, 8, QK(3, 0))
    put2(s2, 3, 0, QK(6, 3))
    put2(s2, 3, 2, QK(7, 0))
    put2(s2, 3, 4, QK(3, 1))

    s3 = mk()
    put2(s3, 1, 0, QK(7, 1))
    put2(s3, 1, 4, QK(3, 2))
    put1(s3, 1, 6, PJ(0, 0))
    put1(s3, 1, 7, PJ(0, 1))
    put2(s3, 2, 0, QK(7, 2))
    put1(s3, 2, 2, PJ(0, 2))
    put1(s3, 2, 3, PJ(0, 3))
    put2(s3, 2, 4, QK(3, 3))
    put1(s3, 2, 6, PJ(0, 4))
    put1(s3, 2, 7, PJ(0, 5))
    put1(s3, 2, 8, PJ(0, 6))
    put1(s3, 2, 9, PJ(0, 7))
    put1(s3, 2, 10, PJ(1, 0))
    put1(s3, 2, 11, PJ(1, 1))
    put2(s3, 3, 0, QK(7, 3))
    for k in range(6):
        put1(s3, 3, 2 + k, PJ(1, 2 + k))
    for k in range(8):
        put1(s3, 3, 8 + k, PJ(2, k))

    attn_pair(0, s0)
    attn_pair(1, s1)
    attn_pair(2, s2)
    attn_pair(3, s3)
    for mt in range(NCHUNK):
        PJ(3, mt)()


def _build_program():
    import contextlib

    import concourse.bass as bass
    import concourse.mybir as mybir
    import concourse.tile as tile
    from concourse import bacc

    nc = bacc.Bacc("TRN2", target_bir_lowering=False, debug=False, num_devices=8)
    f32 = mybir.dt.float32
    bf16 = mybir.dt.bfloat16
    aps = {
        "x": nc.dram_tensor("x", [T, C], bf16, kind="ExternalInput").ap(),
        "wqkv": nc.dram_tensor("wqkv", [C, 3 * GQ], bf16, kind="ExternalInput").ap(),
        "bqk": nc.dram_tensor("bqk", [P, 8], f32, kind="ExternalInput").ap(),
        "bv": nc.dram_tensor("bv", [GQ], f32, kind="ExternalInput").ap(),
        "wp": nc.dram_tensor("wp", [GQ, C], bf16, kind="ExternalInput").ap(),
        "yT": nc.dram_tensor("yT", [C, T], f32, kind="ExternalOutput").ap(),
    }
    with tile.TileContext(nc) as tc:
        with contextlib.ExitStack() as ctx:
            _emit(ctx, tc, aps, mybir, bass)
    nc.compile()
    return nc


def get_program():
    global _PROGRAM
    if _PROGRAM is None:
        _PROGRAM = _build_program()
    return _PROGRAM


def make_in_maps(x, w_qkv, b_qkv, w_proj):
    import ml_dtypes

    bf16 = ml_dtypes.bfloat16
    x = np.asarray(x, np.float32)
    w_qkv = np.asarray(w_qkv, np.float32)
    b_qkv = np.asarray(b_qkv, np.float32)
    w_proj = np.asarray(w_proj, np.float32)
    in_maps = []
    for c in range(8):
        b = c // 2
        g = c % 2
        q0 = g * GQ
        wq = w_qkv[:, q0:q0 + GQ]
        wk = w_qkv[:, C + q0:C + q0 + GQ]
        wv = w_qkv[:, 2 * C + q0:2 * C + q0 + GQ]
        wqkv = np.ascontiguousarray(
            np.concatenate([wq, wk, wv], axis=1).astype(bf16))
        bq = b_qkv[q0:q0 + GQ]
        bk = b_qkv[C + q0:C + q0 + GQ]
        bqk = np.ascontiguousarray(np.concatenate([bq, bk]).reshape(8, P).T)
        bv = np.ascontiguousarray(b_qkv[2 * C + q0:2 * C + q0 + GQ])
        in_maps.append({
            "x": np.ascontiguousarray(x[b].astype(bf16)),
            "wqkv": wqkv,
            "bqk": bqk,
            "bv": bv,
            "wp": np.ascontiguousarray(w_proj[q0:q0 + GQ, :].astype(bf16)),
        })
    return in_maps


def combine_outputs(outs, b_proj):
    b_proj = np.asarray(b_proj, np.float32)
    y = np.empty((B, T, C), np.float32)
    for b in range(B):
        acc = outs[2 * b] + outs[2 * b + 1]  # [C, T]
        y[b] = acc.T + b_proj
    return y


def kernel(x, w_qkv, b_qkv, w_proj, b_proj, _trace=False):
    from concourse import bass_utils

    nc = get_program()
    in_maps = make_in_maps(x, w_qkv, b_qkv, w_proj)
    res = bass_utils.run_bass_kernel_spmd(
        nc, in_maps, core_ids=list(range(8)), trace=_trace
    )
    outs = [r["yT"] for r in res.results]
    y = combine_outputs(outs, b_proj)
    if _trace:
        return y, res
    return y
